# revision 34
# baseline (speedup 1.0000x reference)
"""Trainium2 Bass kernel for segment-mean -> gated-MLP -> gather-gate (nn_Context).

Math (reference):
    seg_sum[s] = sum_{n: bid[n]==s} h_V[n]          # [S, H]
    c_V = seg_sum / max(counts, 1)                  # [S, H]
    hdn = relu(c_V @ W1.T + b1)
    gate = sigmoid(hdn @ W2.T + b2)                 # [S, H]
    out[n] = h_V[n] * gate[bid[n]]                  # [N, H]

Strategy: shard nodes equally across 8 cores. The kernel is DMA-bound
(3 full passes of h_V-sized traffic), so h_V is streamed and the output
written in fp16 (tolerance 2e-2 >> fp16's ~5e-4), and the first
RES_ITERS iteration-tiles of h_V stay resident in SBUF between the two
passes so pass 2 only re-reads the tail. DMAs are 512KB each (every
dma_start costs ~630ns on the serialized HWDGE descriptor generator,
so few big transfers beat many small ones). Per core:
  pass 1: stream the h_V slice in fp16 on the SP queue; per 128-node
          block build a one-hot [128, S] on DVE (is_equal vs iota) and
          accumulate seg_sum [S, H] and counts [S, 1] on the PE into
          PSUM (fp32).
  AllReduce the packed [S, H+1] fp32 partials across the 8 cores (the
  DRAM round-trip rides the ACT queue so it cannot block pass-2
  prefetch on the SP queue during the collective).
  Small replicated MLP on-device: layer 1 via transposed c_V (PE
  transposes + fp16 matmuls), layer 2 computed directly into [S, H]
  with h1-transposed as lhsT (bias via a rank-1 ones matmul), one
  sigmoid writes the fp16 gate. ACT tables are preloaded so no
  LoadActFuncSet lands in this latency chain.
  pass 2: iterations interleave resident/streamed tiles so DMA load is
          uniform (bidr uploaded pre-permuted to match); per pair of
          iterations the 8 gather matmuls [S,128]x[S,H] run
          back-to-back so the PE stays ramped (DVFS). Engine split per
          iteration, respecting that GPSIMD cannot touch PSUM on real
          hardware: block 0 multiplies on DVE straight from PSUM, ACT
          casts blocks 1-3 to fp16 and DVE multiplies them at the
          2x 16-bit rate, Pool (SWDGE) issues the output stores.
"""

import numpy as np

import concourse.bass as bass
import concourse.mybir as mybir
import concourse.tile as tile
from concourse import bacc
from concourse.bass_utils import run_bass_kernel_spmd

N = 262144
H = 512
S = 64
CORES = 8
NPC = N // CORES       # 32768 nodes per core
BLK = 128              # nodes per matmul block (partition dim)
BPI = 4                # blocks per iteration (512KB DMAs; each dma_start
                       # costs ~630ns on the serialized HWDGE device, so
                       # few big transfers beat many small ones)
L = BLK * BPI          # nodes per iteration
ITERS = NPC // L       # 64 iterations per pass
NBLK = NPC // BLK      # 256
KC = H // 128          # 4 column chunks of the hidden dim
RES_ITERS = 36         # leading iterations of h_V kept resident in SBUF
BIDB_B = 2             # pass-2 iterations per batch_id broadcast DMA
F32 = mybir.dt.float32
F16 = mybir.dt.float16


def _pass2_order():
    """Pass-2 processing order: interleave streamed (tail) iterations
    evenly among resident ones so per-iteration DMA traffic is uniform."""
    res = list(range(RES_ITERS))
    tail = list(range(RES_ITERS, ITERS))
    order = []
    ri = ti = 0
    for k in range(ITERS):
        # distribute tail iters at rate len(tail)/ITERS
        if (ti + 1) * ITERS <= (k + 1) * len(tail) and ti < len(tail):
            order.append(tail[ti]); ti += 1
        elif ri < len(res):
            order.append(res[ri]); ri += 1
        else:
            order.append(tail[ti]); ti += 1
    return order

EQ = mybir.AluOpType.is_equal
MULT = mybir.AluOpType.mult
AF = mybir.ActivationFunctionType

_cached = None  # (nc,) built once per process


def _build(use_collective=True, reps=1):
    nc = bacc.Bacc("TRN2", target_bir_lowering=False, debug=False,
                   num_devices=CORES if use_collective else None)

    hv_d = nc.dram_tensor("hv", [NPC, H], F16, kind="ExternalInput")
    bidc_d = nc.dram_tensor("bidc", [BLK, NBLK], F32, kind="ExternalInput")
    bidr_d = nc.dram_tensor("bidr", [NPC], F16, kind="ExternalInput")
    w1t_d = nc.dram_tensor("w1t", [128, KC, H], F16, kind="ExternalInput")
    w2t_d = nc.dram_tensor("w2t", [128, KC, H], F16, kind="ExternalInput")
    b1c_d = nc.dram_tensor("b1c", [128, KC], F32, kind="ExternalInput")
    b2row_d = nc.dram_tensor("b2row", [1, H], F16, kind="ExternalInput")
    ones1_d = nc.dram_tensor("ones1", [1, S], F16, kind="ExternalInput")
    iotar_d = nc.dram_tensor("iotar", [128, S], F32, kind="ExternalInput")
    iotac_d = nc.dram_tensor("iotac", [S, 1], F32, kind="ExternalInput")
    ones_d = nc.dram_tensor("ones", [BLK, 1], F16, kind="ExternalInput")
    ident_d = nc.dram_tensor("ident", [128, 128], F32, kind="ExternalInput")
    out_d = nc.dram_tensor("out", [NPC, H], F16, kind="ExternalOutput")

    # [i, p, b, h]: iteration i, block b, node = i*L + b*BLK + p
    hv_v = hv_d.ap().rearrange("(i b p) h -> i p b h", b=BPI, p=BLK)
    out_v = out_d.ap().rearrange("(i b p) h -> i p b h", b=BPI, p=BLK)
    bidr_ap = bidr_d.ap()

    with tile.TileContext(nc) as tc:
        with (
            tc.tile_pool(name="const", bufs=1) as constp,
            tc.tile_pool(name="resp", bufs=1) as resp,
            tc.tile_pool(name="hvp", bufs=4) as hvp,
            tc.tile_pool(name="ohp", bufs=4) as ohp,
            tc.tile_pool(name="outp", bufs=3) as outp,
            tc.tile_pool(name="smallp", bufs=1) as smallp,
            tc.tile_pool(name="dramp", bufs=1, space="DRAM") as dramp,
        ):
            # ---- constants ----
            bidc = constp.tile([BLK, NBLK], F32, tag="bidc")
            nc.scalar.dma_start(out=bidc[:], in_=bidc_d.ap())
            iotar = constp.tile([128, S], F32, tag="iotar")
            nc.scalar.dma_start(out=iotar[:], in_=iotar_d.ap())
            iotac = constp.tile([S, 1], F32, tag="iotac")
            nc.scalar.dma_start(out=iotac[:], in_=iotac_d.ap())
            ones = constp.tile([BLK, 1], F16, tag="ones")
            nc.scalar.dma_start(out=ones[:], in_=ones_d.ap())
            ident = constp.tile([128, 128], F32, tag="ident")
            nc.scalar.dma_start(out=ident[:], in_=ident_d.ap())
            w1t = constp.tile([128, KC, H], F16, tag="w1t")
            nc.scalar.dma_start(out=w1t[:], in_=w1t_d.ap())
            w2t = constp.tile([128, KC, H], F16, tag="w2t")
            nc.scalar.dma_start(out=w2t[:], in_=w2t_d.ap())
            b1c = constp.tile([128, KC], F32, tag="b1c")
            nc.scalar.dma_start(out=b1c[:], in_=b1c_d.ap())
            b2row = constp.tile([1, H], F16, tag="b2row")
            nc.scalar.dma_start(out=b2row[:], in_=b2row_d.ap())
            ones1 = constp.tile([1, S], F16, tag="ones1")
            nc.scalar.dma_start(out=ones1[:], in_=ones1_d.ap())
            gate = constp.tile([S, H], F16, tag="gate")
            res = resp.tile([BLK, RES_ITERS, BPI, H], F16, tag="res")

            # Preload the ACT function tables (Relu/Sigmoid) with dummy ops
            # so the ~1.3us LoadActFuncSet doesn't land in the post-collective
            # latency chain.
            warm = constp.tile([1, 2], F32, tag="warm")
            nc.scalar.activation(warm[:, 0:1], warm[:, 1:2], AF.Relu,
                                 bias=b1c[:1, 0:1])
            nc.scalar.activation(warm[:, 0:1], warm[:, 1:2], AF.Sigmoid,
                                 bias=b1c[:1, 0:1])

            def body():
                _body(nc, tc, hv_v, out_v, bidr_ap, bidc, iotar, iotac, ones,
                      ident, w1t, w2t, b1c, b2row, ones1, gate, res, hvp,
                      ohp, outp, smallp, dramp, use_collective)

            if reps == 1:
                body()
            else:
                with tc.For_i(0, reps, 1):
                    body()

    nc.compile()
    return nc


def _body(nc, tc, hv_v, out_v, bidr_ap, bidc, iotar, iotac, ones, ident,
          w1t, w2t, b1c, b2row, ones1, gate, res, hvp, ohp, outp, smallp,
          dramp, use_collective):
    with tc.tile_pool(name="psacc", bufs=1, space="PSUM") as psacc, \
         tc.tile_pool(name="psmlp", bufs=2, space="PSUM") as psmlp:
        # ---- pass 1: per-core seg_sum [S, H] and counts [S, 1] ----
        pseg = psacc.tile([S, H], F32, tag="pseg")
        pcnt = psacc.tile([S, 1], F32, tag="pcnt")

        for j in range(ITERS):
            if j < RES_ITERS:
                hv_t = res[:, j]
            else:
                hv_t = hvp.tile([BLK, BPI, H], F16, tag="hv")
            nc.sync.dma_start(out=hv_t[:], in_=hv_v[j])
            for b in range(BPI):
                i = j * BPI + b
                oh = ohp.tile([BLK, S], F16, tag="oh")
                nc.vector.tensor_scalar(
                    out=oh[:], in0=iotar[:],
                    scalar1=bidc[:, i:i + 1], scalar2=None, op0=EQ)
                first = i == 0
                last = i == NBLK - 1
                nc.tensor.matmul(pseg[:], lhsT=oh[:],
                                 rhs=hv_t[:, b, :],
                                 start=first, stop=last)
                nc.tensor.matmul(pcnt[:], lhsT=oh[:], rhs=ones[:],
                                 start=first, stop=last)

        # ---- AllReduce partial stats across the 8 cores ----
        pack = smallp.tile([S, H + 1], F32, tag="pack")
        nc.scalar.copy(pack[:, :H], pseg[:])
        nc.scalar.copy(pack[:, H:H + 1], pcnt[:])
        # The collective round-trip stays off the sync (load) queue so the
        # in-order queue cannot block pass-2 prefetch during the bubble;
        # the ACT queue only carries MLP work here, which depends on the
        # collective anyway.
        cc_in = dramp.tile([S, H + 1], F32, tag="ccin")
        cc_out = dramp.tile([S, H + 1], F32, tag="ccout")
        nc.scalar.dma_start(out=cc_in[:], in_=pack[:])
        if use_collective:
            nc.gpsimd.collective_compute(
                "AllReduce",
                mybir.AluOpType.add,
                replica_groups=[list(range(CORES))],
                ins=[cc_in[:].opt()],
                outs=[cc_out[:].opt()],
            )
        else:  # single-core timing-model variant
            nc.gpsimd.dma_start(out=cc_out[:], in_=cc_in[:])
        packr = smallp.tile([S, H + 1], F32, tag="packr")
        nc.scalar.dma_start(out=packr[:], in_=cc_out[:])

        # ---- c_V = seg_sum / max(counts, 1) ----
        cnt = smallp.tile([S, 1], F32, tag="cnt")
        nc.vector.tensor_scalar_max(cnt[:], packr[:, H:H + 1], 1.0)
        rcp = smallp.tile([S, 1], F32, tag="rcp")
        nc.vector.reciprocal(rcp[:], cnt[:])
        cv = smallp.tile([S, H], F32, tag="cv")
        nc.vector.tensor_scalar_mul(cv[:], packr[:, :H], rcp[:])

        # ---- transpose c_V -> ct [128, kc, S] (k on partitions) ----
        ct = smallp.tile([128, KC, S], F16, tag="ct")
        for kc in range(KC):
            pt = psmlp.tile([128, S], F32, tag="mlp")
            nc.tensor.transpose(pt[:], in_=cv[:, kc * 128:(kc + 1) * 128],
                                identity=ident[:S, :S])
            nc.scalar.copy(ct[:, kc, :], pt[:])

        # ---- layer 1: h1_T[j, s] = relu(W1 @ c_V.T + b1) ----
        h1 = smallp.tile([128, KC, S], F16, tag="h1")
        for jc in range(KC):
            ph = psmlp.tile([128, S], F32, tag="mlp")
            for kc in range(KC):
                nc.tensor.matmul(
                    ph[:], lhsT=w1t[:, kc, jc * 128:(jc + 1) * 128],
                    rhs=ct[:, kc, :], start=kc == 0, stop=kc == KC - 1)
            nc.scalar.activation(h1[:, jc, :], ph[:], AF.Relu,
                                 bias=b1c[:, jc:jc + 1])

        # ---- layer 2 direct: gate[s, h] = sigmoid(sum_j h1[j,s] W2T[j,h]
        # + b2[h]), computed straight into [S, H] layout (h1 already holds
        # hdn transposed, and w2t[:, jc, :] is W2.T rows) -- no transposes.
        # The bias lands via a rank-1 matmul with a ones column.
        pgs = psmlp.tile([S, H], F32, tag="mlpg")
        for jc in range(KC):
            nc.tensor.matmul(
                pgs[:], lhsT=h1[:, jc, :], rhs=w2t[:, jc, :],
                start=jc == 0, stop=False)
        nc.tensor.matmul(pgs[:], lhsT=ones1[:], rhs=b2row[:],
                         start=False, stop=True)
        nc.scalar.activation(gate[:], pgs[:], AF.Sigmoid)

    # ---- pass 2: out = h_V * gate[bid] ----
    # Iterations are processed in an order that interleaves streamed (tail)
    # and SBUF-resident iterations so the DMA load stays even; bidr is
    # uploaded pre-permuted to match (see _pass2_order / _prep_inputs).
    # batch_id rows for BIDB_B iterations are broadcast in one DMA on the
    # SP (HWDGE) queue -- gpsimd.dma_start would run SWDGE descriptor
    # generation on the Pool engine (~1us per op).
    order = _pass2_order()
    with tc.tile_pool(name="psg", bufs=8, space="PSUM") as psg:
        # Iterations run in PAIRS: both iterations' 8 gather matmuls are
        # emitted back-to-back so the PE stays ramped (its DVFS model runs
        # matmuls ~2-4x slower right after an idle period); the 8 PSUM
        # banks hold exactly one pair.
        for k0 in range(0, ITERS, 2):
            pair = [(k0 + d, order[k0 + d]) for d in range(2)]
            bidb = ohp.tile([S, BIDB_B * L], F16, tag="bidb", bufs=2)
            nc.sync.dma_start(
                out=bidb[:],
                in_=bidr_ap[k0 * L:(k0 + BIDB_B) * L].partition_broadcast(S))
            ohts, hvts = [], []
            for d, (k, j) in enumerate(pair):
                oht = ohp.tile([S, L], F16, tag="oht")
                nc.vector.tensor_scalar(
                    out=oht[:], in0=bidb[:, d * L:(d + 1) * L],
                    scalar1=iotac[:], scalar2=None, op0=EQ)
                ohts.append(oht)
                if j < RES_ITERS:
                    hvts.append(res[:, j])
                else:
                    # two half-loads spread the DMA work evenly across
                    # pairs; subtile deps let blocks 0-1 proceed off the
                    # first half
                    hv_t = hvp.tile([BLK, BPI, H], F16, tag="hv")
                    nc.sync.dma_start(out=hv_t[:, :BPI // 2],
                                      in_=hv_v[j][:, :BPI // 2])
                    nc.sync.dma_start(out=hv_t[:, BPI // 2:],
                                      in_=hv_v[j][:, BPI // 2:])
                    hvts.append(hv_t)
            pgbs = []
            for d, (k, j) in enumerate(pair):
                for b in range(BPI):
                    pgb = psg.tile([BLK, H], F32, tag="pg2")
                    nc.tensor.matmul(
                        pgb[:],
                        lhsT=ohts[d][:, b * BLK:(b + 1) * BLK],
                        rhs=gate[:], start=True, stop=True)
                    pgbs.append(pgb)
            # The four multiplies per iteration are spread so no engine
            # exceeds the per-iteration DMA budget (~2.2us): blocks 0-1 DVE
            # straight from PSUM (818ns each), block 2 Pool (1173ns),
            # block 3 ACT-casts the PSUM gate rows to fp16 (831ns) and DVE
            # multiplies fp16 x fp16 at the 2x rate (327ns). One store per
            # iteration.
            for d, (k, j) in enumerate(pair):
                hv_t = hvts[d]
                ot = outp.tile([BLK, BPI, H], F16, tag="ot")
                for b in range(BPI):
                    pgb = pgbs[d * BPI + b]
                    # GPSIMD cannot touch PSUM on real hardware, so Pool
                    # issues the stores (SWDGE) instead of multiplying.
                    # Block 0 multiplies on DVE straight from PSUM; for
                    # blocks 1-3 ACT casts the PSUM gate rows to fp16 and
                    # DVE multiplies fp16 x fp16 at the 2x rate -- DVE and
                    # ACT both stay under the per-iteration DMA budget.
                    if b == 0:
                        nc.vector.tensor_tensor(out=ot[:, b, :],
                                                in0=hv_t[:, b, :],
                                                in1=pgb[:], op=MULT)
                    else:
                        gc = ohp.tile([BLK, H], F16, tag="gc", bufs=6)
                        nc.scalar.copy(gc[:], pgb[:])
                        nc.vector.tensor_tensor(out=ot[:, b, :],
                                                in0=hv_t[:, b, :],
                                                in1=gc[:], op=MULT)
                nc.gpsimd.dma_start(out=out_v[j], in_=ot[:])


def _prep_inputs(inputs):
    h_V = np.asarray(inputs["h_V"], dtype=np.float32)
    bid = np.asarray(inputs["batch_id"])
    W1 = np.asarray(inputs["W1"], dtype=np.float32)
    b1 = np.asarray(inputs["b1"], dtype=np.float32)
    W2 = np.asarray(inputs["W2"], dtype=np.float32)
    b2 = np.asarray(inputs["b2"], dtype=np.float32)

    h_V16 = np.ascontiguousarray(h_V.astype(np.float16))
    bid_f = bid.astype(np.float32)
    bid_h = bid.astype(np.float16)
    w1t = np.ascontiguousarray(
        W1.T.reshape(KC, 128, H).transpose(1, 0, 2)).astype(np.float16)
    w2t = np.ascontiguousarray(
        W2.T.reshape(KC, 128, H).transpose(1, 0, 2)).astype(np.float16)
    b1c = np.ascontiguousarray(b1.reshape(KC, 128).T)
    b2row = b2.reshape(1, H).astype(np.float16)
    ones1 = np.ones((1, S), dtype=np.float16)
    iotar = np.ascontiguousarray(
        np.tile(np.arange(S, dtype=np.float32), (128, 1)))
    iotac = np.arange(S, dtype=np.float32).reshape(S, 1)
    ones = np.ones((BLK, 1), dtype=np.float16)
    ident = np.eye(128, dtype=np.float32)

    order = np.asarray(_pass2_order())
    in_maps = []
    for c in range(CORES):
        lo, hi = c * NPC, (c + 1) * NPC
        # bidr is consumed in pass-2 processing order, L ids per iteration
        bid_c = bid_h[lo:hi].reshape(ITERS, L)[order].reshape(-1)
        in_maps.append({
            "hv": h_V16[lo:hi],
            "bidc": np.ascontiguousarray(
                bid_f[lo:hi].reshape(NBLK, BLK).T),
            "bidr": np.ascontiguousarray(bid_c),
            "w1t": w1t, "w2t": w2t, "b1c": b1c, "b2row": b2row,
            "ones1": ones1,
            "iotar": iotar, "iotac": iotac, "ones": ones, "ident": ident,
        })
    return in_maps


def _run(inputs, trace=False):
    global _cached
    if _cached is None:
        _cached = _build()
    nc = _cached
    in_maps = _prep_inputs(inputs)
    res = run_bass_kernel_spmd(nc, in_maps, core_ids=list(range(CORES)),
                               trace=trace)
    out = np.concatenate(
        [res.results[c]["out"] for c in range(CORES)], axis=0)
    return np.ascontiguousarray(out.astype(np.float32)), res


def kernel(**inputs) -> np.ndarray:
    out, _ = _run(inputs, trace=False)
    return out


# revision 35
# speedup vs baseline: 1.0551x; 1.0551x over previous
"""Trainium2 Bass kernel for segment-mean -> gated-MLP -> gather-gate (nn_Context).

Math (reference):
    seg_sum[s] = sum_{n: bid[n]==s} h_V[n]          # [S, H]
    c_V = seg_sum / max(counts, 1)                  # [S, H]
    hdn = relu(c_V @ W1.T + b1)
    gate = sigmoid(hdn @ W2.T + b2)                 # [S, H]
    out[n] = h_V[n] * gate[bid[n]]                  # [N, H]

Strategy: shard nodes equally across 8 cores. The kernel is DMA-bound
(3 full passes of h_V-sized traffic), so h_V is streamed and the output
written in fp16 (tolerance 2e-2 >> fp16's ~5e-4), and the first
RES_ITERS iteration-tiles of h_V stay resident in SBUF between the two
passes so pass 2 only re-reads the tail. DMAs are 512KB each (every
dma_start costs ~630ns on the serialized HWDGE descriptor generator,
so few big transfers beat many small ones). Per core:
  pass 1: stream the h_V slice in fp16 on the SP queue; per 128-node
          block build a one-hot [128, S] on DVE (is_equal vs iota) and
          accumulate seg_sum [S, H] and counts [S, 1] on the PE into
          PSUM (fp32).
  AllReduce the packed [S, H+1] fp32 partials across the 8 cores (the
  DRAM round-trip rides the ACT queue so it cannot block pass-2
  prefetch on the SP queue during the collective).
  Small replicated MLP on-device: layer 1 via transposed c_V (PE
  transposes + fp16 matmuls), layer 2 computed directly into [S, H]
  with h1-transposed as lhsT (bias via a rank-1 ones matmul), one
  sigmoid writes the fp16 gate. ACT tables are preloaded so no
  LoadActFuncSet lands in this latency chain.
  pass 2: iterations interleave resident/streamed tiles so DMA load is
          uniform (bidr uploaded pre-permuted to match); per pair of
          iterations the 8 gather matmuls [S,128]x[S,H] run
          back-to-back so the PE stays ramped (DVFS). Engine split per
          iteration, respecting that GPSIMD cannot touch PSUM on real
          hardware: block 0 multiplies on DVE straight from PSUM, ACT
          casts blocks 1-3 to fp16 and DVE multiplies them at the
          2x 16-bit rate, Pool (SWDGE) issues the output stores.
"""

import numpy as np

import concourse.bass as bass
import concourse.mybir as mybir
import concourse.tile as tile
from concourse import bacc
from concourse.bass_utils import run_bass_kernel_spmd

N = 262144
H = 512
S = 64
CORES = 8
NPC = N // CORES       # 32768 nodes per core
BLK = 128              # nodes per matmul block (partition dim)
BPI = 4                # blocks per iteration (512KB DMAs; each dma_start
                       # costs ~630ns on the serialized HWDGE device, so
                       # few big transfers beat many small ones)
L = BLK * BPI          # nodes per iteration
ITERS = NPC // L       # 64 iterations per pass
NBLK = NPC // BLK      # 256
KC = H // 128          # 4 column chunks of the hidden dim
RES_ITERS = 36         # leading iterations of h_V kept resident in SBUF
BIDB_B = 2             # pass-2 iterations per batch_id broadcast DMA
F32 = mybir.dt.float32
F16 = mybir.dt.float16
F8 = mybir.dt.float8e4


def _pass2_order():
    """Pass-2 processing order: interleave streamed (tail) iterations
    evenly among resident ones so per-iteration DMA traffic is uniform."""
    res = list(range(RES_ITERS))
    tail = list(range(RES_ITERS, ITERS))
    order = []
    ri = ti = 0
    for k in range(ITERS):
        # distribute tail iters at rate len(tail)/ITERS
        if (ti + 1) * ITERS <= (k + 1) * len(tail) and ti < len(tail):
            order.append(tail[ti]); ti += 1
        elif ri < len(res):
            order.append(res[ri]); ri += 1
        else:
            order.append(tail[ti]); ti += 1
    return order

EQ = mybir.AluOpType.is_equal
MULT = mybir.AluOpType.mult
AF = mybir.ActivationFunctionType

_cached = None  # (nc,) built once per process


def _build(use_collective=True, reps=1):
    nc = bacc.Bacc("TRN2", target_bir_lowering=False, debug=False,
                   num_devices=CORES if use_collective else None)

    hv_d = nc.dram_tensor("hv", [NPC, H], F16, kind="ExternalInput")
    hv8_d = nc.dram_tensor("hv8", [NPC, H], F8, kind="ExternalInput")
    bidc_d = nc.dram_tensor("bidc", [BLK, NBLK], F32, kind="ExternalInput")
    bidr_d = nc.dram_tensor("bidr", [NPC], F16, kind="ExternalInput")
    w1t_d = nc.dram_tensor("w1t", [128, KC, H], F16, kind="ExternalInput")
    w2t_d = nc.dram_tensor("w2t", [128, KC, H], F16, kind="ExternalInput")
    b1c_d = nc.dram_tensor("b1c", [128, KC], F32, kind="ExternalInput")
    b2row_d = nc.dram_tensor("b2row", [1, H], F16, kind="ExternalInput")
    ones1_d = nc.dram_tensor("ones1", [1, S], F16, kind="ExternalInput")
    iotar_d = nc.dram_tensor("iotar", [128, S], F32, kind="ExternalInput")
    iotac_d = nc.dram_tensor("iotac", [S, 1], F32, kind="ExternalInput")
    ones_d = nc.dram_tensor("ones", [BLK, 1], F16, kind="ExternalInput")
    ident_d = nc.dram_tensor("ident", [128, 128], F32, kind="ExternalInput")
    out_d = nc.dram_tensor("out", [NPC, H], F16, kind="ExternalOutput")

    # [i, p, b, h]: iteration i, block b, node = i*L + b*BLK + p
    hv_v = hv_d.ap().rearrange("(i b p) h -> i p b h", b=BPI, p=BLK)
    hv8_v = hv8_d.ap().rearrange("(i b p) h -> i p b h", b=BPI, p=BLK)
    out_v = out_d.ap().rearrange("(i b p) h -> i p b h", b=BPI, p=BLK)
    bidr_ap = bidr_d.ap()

    with tile.TileContext(nc) as tc:
        with (
            tc.tile_pool(name="const", bufs=1) as constp,
            tc.tile_pool(name="resp", bufs=1) as resp,
            tc.tile_pool(name="hvp", bufs=4) as hvp,
            tc.tile_pool(name="ohp", bufs=4) as ohp,
            tc.tile_pool(name="outp", bufs=3) as outp,
            tc.tile_pool(name="smallp", bufs=1) as smallp,
            tc.tile_pool(name="dramp", bufs=1, space="DRAM") as dramp,
        ):
            # ---- constants ----
            bidc = constp.tile([BLK, NBLK], F32, tag="bidc")
            nc.scalar.dma_start(out=bidc[:], in_=bidc_d.ap())
            iotar = constp.tile([128, S], F32, tag="iotar")
            nc.scalar.dma_start(out=iotar[:], in_=iotar_d.ap())
            iotac = constp.tile([S, 1], F32, tag="iotac")
            nc.scalar.dma_start(out=iotac[:], in_=iotac_d.ap())
            ones = constp.tile([BLK, 1], F16, tag="ones")
            nc.scalar.dma_start(out=ones[:], in_=ones_d.ap())
            ones8 = constp.tile([BLK, 1], F8, tag="ones8")
            nc.vector.memset(ones8[:], 1.0)
            ident = constp.tile([128, 128], F32, tag="ident")
            nc.scalar.dma_start(out=ident[:], in_=ident_d.ap())
            w1t = constp.tile([128, KC, H], F16, tag="w1t")
            nc.scalar.dma_start(out=w1t[:], in_=w1t_d.ap())
            w2t = constp.tile([128, KC, H], F16, tag="w2t")
            nc.scalar.dma_start(out=w2t[:], in_=w2t_d.ap())
            b1c = constp.tile([128, KC], F32, tag="b1c")
            nc.scalar.dma_start(out=b1c[:], in_=b1c_d.ap())
            b2row = constp.tile([1, H], F16, tag="b2row")
            nc.scalar.dma_start(out=b2row[:], in_=b2row_d.ap())
            ones1 = constp.tile([1, S], F16, tag="ones1")
            nc.scalar.dma_start(out=ones1[:], in_=ones1_d.ap())
            gate = constp.tile([S, H], F16, tag="gate")
            res = resp.tile([BLK, RES_ITERS, BPI, H], F16, tag="res")

            # Preload the ACT function tables (Relu/Sigmoid) with dummy ops
            # so the ~1.3us LoadActFuncSet doesn't land in the post-collective
            # latency chain.
            warm = constp.tile([1, 2], F32, tag="warm")
            nc.scalar.activation(warm[:, 0:1], warm[:, 1:2], AF.Relu,
                                 bias=b1c[:1, 0:1])
            nc.scalar.activation(warm[:, 0:1], warm[:, 1:2], AF.Sigmoid,
                                 bias=b1c[:1, 0:1])

            def body():
                _body(nc, tc, hv_v, hv8_v, out_v, bidr_ap, bidc, iotar,
                      iotac, ones, ones8, ident, w1t, w2t, b1c, b2row,
                      ones1, gate, res, hvp, ohp, outp, smallp, dramp,
                      use_collective)

            if reps == 1:
                body()
            else:
                with tc.For_i(0, reps, 1):
                    body()

    nc.compile()
    return nc


def _body(nc, tc, hv_v, hv8_v, out_v, bidr_ap, bidc, iotar, iotac, ones,
          ones8, ident, w1t, w2t, b1c, b2row, ones1, gate, res, hvp, ohp,
          outp, smallp, dramp, use_collective):
    with tc.tile_pool(name="psacc", bufs=1, space="PSUM") as psacc, \
         tc.tile_pool(name="psmlp", bufs=2, space="PSUM") as psmlp:
        # ---- pass 1: per-core seg_sum [S, H] and counts [S, 1] ----
        pseg = psacc.tile([S, H], F32, tag="pseg")
        pcnt = psacc.tile([S, 1], F32, tag="pcnt")

        # Resident iterations load fp16 (reused by pass 2); tail
        # iterations load an fp8 copy instead -- they are re-read in fp16
        # by pass 2 anyway, and the segment MEAN over ~4k nodes shrinks
        # fp8's ~3% element error to ~0.1%. Halves pass-1 tail traffic.
        for j in range(ITERS):
            if j < RES_ITERS:
                hv_t = res[:, j]
                nc.sync.dma_start(out=hv_t[:], in_=hv_v[j])
                mdt, onesd = F16, ones
            else:
                hv_t = hvp.tile([BLK, BPI, H], F8, tag="hv")
                nc.sync.dma_start(out=hv_t[:], in_=hv8_v[j])
                mdt, onesd = F8, ones8
            for b in range(BPI):
                i = j * BPI + b
                oh = ohp.tile([BLK, S], mdt, tag="oh")
                nc.vector.tensor_scalar(
                    out=oh[:], in0=iotar[:],
                    scalar1=bidc[:, i:i + 1], scalar2=None, op0=EQ)
                first = i == 0
                last = i == NBLK - 1
                nc.tensor.matmul(pseg[:], lhsT=oh[:],
                                 rhs=hv_t[:, b, :],
                                 start=first, stop=last)
                nc.tensor.matmul(pcnt[:], lhsT=oh[:], rhs=onesd[:],
                                 start=first, stop=last)

        # ---- AllReduce partial stats across the 8 cores ----
        pack = smallp.tile([S, H + 1], F32, tag="pack")
        nc.scalar.copy(pack[:, :H], pseg[:])
        nc.scalar.copy(pack[:, H:H + 1], pcnt[:])
        # The collective round-trip stays off the sync (load) queue so the
        # in-order queue cannot block pass-2 prefetch during the bubble;
        # the ACT queue only carries MLP work here, which depends on the
        # collective anyway.
        cc_in = dramp.tile([S, H + 1], F32, tag="ccin")
        cc_out = dramp.tile([S, H + 1], F32, tag="ccout")
        nc.scalar.dma_start(out=cc_in[:], in_=pack[:])
        if use_collective:
            nc.gpsimd.collective_compute(
                "AllReduce",
                mybir.AluOpType.add,
                replica_groups=[list(range(CORES))],
                ins=[cc_in[:].opt()],
                outs=[cc_out[:].opt()],
            )
        else:  # single-core timing-model variant
            nc.gpsimd.dma_start(out=cc_out[:], in_=cc_in[:])
        packr = smallp.tile([S, H + 1], F32, tag="packr")
        nc.scalar.dma_start(out=packr[:], in_=cc_out[:])

        # ---- c_V = seg_sum / max(counts, 1) ----
        cnt = smallp.tile([S, 1], F32, tag="cnt")
        nc.vector.tensor_scalar_max(cnt[:], packr[:, H:H + 1], 1.0)
        rcp = smallp.tile([S, 1], F32, tag="rcp")
        nc.vector.reciprocal(rcp[:], cnt[:])
        cv = smallp.tile([S, H], F32, tag="cv")
        nc.vector.tensor_scalar_mul(cv[:], packr[:, :H], rcp[:])

        # ---- transpose c_V -> ct [128, kc, S] (k on partitions) ----
        ct = smallp.tile([128, KC, S], F16, tag="ct")
        for kc in range(KC):
            pt = psmlp.tile([128, S], F32, tag="mlp")
            nc.tensor.transpose(pt[:], in_=cv[:, kc * 128:(kc + 1) * 128],
                                identity=ident[:S, :S])
            nc.scalar.copy(ct[:, kc, :], pt[:])

        # ---- layer 1: h1_T[j, s] = relu(W1 @ c_V.T + b1) ----
        h1 = smallp.tile([128, KC, S], F16, tag="h1")
        for jc in range(KC):
            ph = psmlp.tile([128, S], F32, tag="mlp")
            for kc in range(KC):
                nc.tensor.matmul(
                    ph[:], lhsT=w1t[:, kc, jc * 128:(jc + 1) * 128],
                    rhs=ct[:, kc, :], start=kc == 0, stop=kc == KC - 1)
            nc.scalar.activation(h1[:, jc, :], ph[:], AF.Relu,
                                 bias=b1c[:, jc:jc + 1])

        # ---- layer 2 direct: gate[s, h] = sigmoid(sum_j h1[j,s] W2T[j,h]
        # + b2[h]), computed straight into [S, H] layout (h1 already holds
        # hdn transposed, and w2t[:, jc, :] is W2.T rows) -- no transposes.
        # The bias lands via a rank-1 matmul with a ones column.
        pgs = psmlp.tile([S, H], F32, tag="mlpg")
        for jc in range(KC):
            nc.tensor.matmul(
                pgs[:], lhsT=h1[:, jc, :], rhs=w2t[:, jc, :],
                start=jc == 0, stop=False)
        nc.tensor.matmul(pgs[:], lhsT=ones1[:], rhs=b2row[:],
                         start=False, stop=True)
        nc.scalar.activation(gate[:], pgs[:], AF.Sigmoid)

    # ---- pass 2: out = h_V * gate[bid] ----
    # Iterations are processed in an order that interleaves streamed (tail)
    # and SBUF-resident iterations so the DMA load stays even; bidr is
    # uploaded pre-permuted to match (see _pass2_order / _prep_inputs).
    # batch_id rows for BIDB_B iterations are broadcast in one DMA on the
    # SP (HWDGE) queue -- gpsimd.dma_start would run SWDGE descriptor
    # generation on the Pool engine (~1us per op).
    order = _pass2_order()
    with tc.tile_pool(name="psg", bufs=8, space="PSUM") as psg:
        # Iterations run in PAIRS: both iterations' 8 gather matmuls are
        # emitted back-to-back so the PE stays ramped (its DVFS model runs
        # matmuls ~2-4x slower right after an idle period); the 8 PSUM
        # banks hold exactly one pair.
        for k0 in range(0, ITERS, 2):
            pair = [(k0 + d, order[k0 + d]) for d in range(2)]
            bidb = ohp.tile([S, BIDB_B * L], F16, tag="bidb", bufs=2)
            nc.sync.dma_start(
                out=bidb[:],
                in_=bidr_ap[k0 * L:(k0 + BIDB_B) * L].partition_broadcast(S))
            ohts, hvts = [], []
            for d, (k, j) in enumerate(pair):
                oht = ohp.tile([S, L], F16, tag="oht")
                nc.vector.tensor_scalar(
                    out=oht[:], in0=bidb[:, d * L:(d + 1) * L],
                    scalar1=iotac[:], scalar2=None, op0=EQ)
                ohts.append(oht)
                if j < RES_ITERS:
                    hvts.append(res[:, j])
                else:
                    # two half-loads spread the DMA work evenly across
                    # pairs; subtile deps let blocks 0-1 proceed off the
                    # first half
                    hv_t = hvp.tile([BLK, BPI, H], F16, tag="hv")
                    nc.sync.dma_start(out=hv_t[:, :BPI // 2],
                                      in_=hv_v[j][:, :BPI // 2])
                    nc.sync.dma_start(out=hv_t[:, BPI // 2:],
                                      in_=hv_v[j][:, BPI // 2:])
                    hvts.append(hv_t)
            pgbs = []
            for d, (k, j) in enumerate(pair):
                for b in range(BPI):
                    pgb = psg.tile([BLK, H], F32, tag="pg2")
                    nc.tensor.matmul(
                        pgb[:],
                        lhsT=ohts[d][:, b * BLK:(b + 1) * BLK],
                        rhs=gate[:], start=True, stop=True)
                    pgbs.append(pgb)
            # The four multiplies per iteration are spread so no engine
            # exceeds the per-iteration DMA budget (~2.2us): blocks 0-1 DVE
            # straight from PSUM (818ns each), block 2 Pool (1173ns),
            # block 3 ACT-casts the PSUM gate rows to fp16 (831ns) and DVE
            # multiplies fp16 x fp16 at the 2x rate (327ns). One store per
            # iteration.
            for d, (k, j) in enumerate(pair):
                hv_t = hvts[d]
                ot = outp.tile([BLK, BPI, H], F16, tag="ot")
                for b in range(BPI):
                    pgb = pgbs[d * BPI + b]
                    # GPSIMD cannot touch PSUM on real hardware, so Pool
                    # issues the stores (SWDGE) instead of multiplying.
                    # Block 0 multiplies on DVE straight from PSUM; for
                    # blocks 1-3 ACT casts the PSUM gate rows to fp16 and
                    # DVE multiplies fp16 x fp16 at the 2x rate -- DVE and
                    # ACT both stay under the per-iteration DMA budget.
                    if b == 0:
                        nc.vector.tensor_tensor(out=ot[:, b, :],
                                                in0=hv_t[:, b, :],
                                                in1=pgb[:], op=MULT)
                    else:
                        gc = ohp.tile([BLK, H], F16, tag="gc", bufs=6)
                        nc.scalar.copy(gc[:], pgb[:])
                        nc.vector.tensor_tensor(out=ot[:, b, :],
                                                in0=hv_t[:, b, :],
                                                in1=gc[:], op=MULT)
                nc.gpsimd.dma_start(out=out_v[j], in_=ot[:])


def _prep_inputs(inputs):
    h_V = np.asarray(inputs["h_V"], dtype=np.float32)
    bid = np.asarray(inputs["batch_id"])
    W1 = np.asarray(inputs["W1"], dtype=np.float32)
    b1 = np.asarray(inputs["b1"], dtype=np.float32)
    W2 = np.asarray(inputs["W2"], dtype=np.float32)
    b2 = np.asarray(inputs["b2"], dtype=np.float32)

    import ml_dtypes
    h_V16 = np.ascontiguousarray(h_V.astype(np.float16))
    h_V8 = np.ascontiguousarray(h_V.astype(ml_dtypes.float8_e4m3fn))
    bid_f = bid.astype(np.float32)
    bid_h = bid.astype(np.float16)
    w1t = np.ascontiguousarray(
        W1.T.reshape(KC, 128, H).transpose(1, 0, 2)).astype(np.float16)
    w2t = np.ascontiguousarray(
        W2.T.reshape(KC, 128, H).transpose(1, 0, 2)).astype(np.float16)
    b1c = np.ascontiguousarray(b1.reshape(KC, 128).T)
    b2row = b2.reshape(1, H).astype(np.float16)
    ones1 = np.ones((1, S), dtype=np.float16)
    iotar = np.ascontiguousarray(
        np.tile(np.arange(S, dtype=np.float32), (128, 1)))
    iotac = np.arange(S, dtype=np.float32).reshape(S, 1)
    ones = np.ones((BLK, 1), dtype=np.float16)
    ident = np.eye(128, dtype=np.float32)

    order = np.asarray(_pass2_order())
    in_maps = []
    for c in range(CORES):
        lo, hi = c * NPC, (c + 1) * NPC
        # bidr is consumed in pass-2 processing order, L ids per iteration
        bid_c = bid_h[lo:hi].reshape(ITERS, L)[order].reshape(-1)
        in_maps.append({
            "hv": h_V16[lo:hi],
            "hv8": h_V8[lo:hi],
            "bidc": np.ascontiguousarray(
                bid_f[lo:hi].reshape(NBLK, BLK).T),
            "bidr": np.ascontiguousarray(bid_c),
            "w1t": w1t, "w2t": w2t, "b1c": b1c, "b2row": b2row,
            "ones1": ones1,
            "iotar": iotar, "iotac": iotac, "ones": ones, "ident": ident,
        })
    return in_maps


def _run(inputs, trace=False):
    global _cached
    if _cached is None:
        _cached = _build()
    nc = _cached
    in_maps = _prep_inputs(inputs)
    res = run_bass_kernel_spmd(nc, in_maps, core_ids=list(range(CORES)),
                               trace=trace)
    out = np.concatenate(
        [res.results[c]["out"] for c in range(CORES)], axis=0)
    return np.ascontiguousarray(out.astype(np.float32)), res


def kernel(**inputs) -> np.ndarray:
    out, _ = _run(inputs, trace=False)
    return out
